# revision 22
# baseline (speedup 1.0000x reference)
"""MinGRU layer Trainium2 kernel.

Reference semantics (B=8, T=16384, D=H=O=256):
    zs = sigmoid(xs @ Wz.T + bz);  hs = xs @ Wh.T + bh
    a = concat([1], 1-zs);  b = concat([0], zs*hs)         (T+1 positions)
    states = jax.lax.associative_scan(combine, (a, b))[1][:, 1:]
    out = states @ Wo.T + bo
with combine((a0,b0),(a1,b1)) = (a0*b0, b0*a1 + b1).

The combine is NOT associative, so the result is defined by jax's exact
odd/even recursion tree.  We replicate that tree exactly:
  - positions split into 8 aligned chunks of L=2048 (+1 trailing position);
  - per-chunk bottom-up reduce ("up-sweep", keeping all tree levels);
  - a tiny cross-chunk scan over the 8 chunk-top elements following the same
    recursion (chunk prefixes + chunk-end outputs);
  - per-chunk top-down "down-sweep" filling every position's scan value.

Sharding: batch b=8 across the 8 cores (one sequence per core); weights
replicated.  The host pre-transposes/casts x and the weights; the device does
DMA -> matmul (bf16) -> sigmoid (ACT) -> scan tree (DVE+GpSimd, f32, both
hidden halves fused per op via 3D APs) -> output matmul (f32r, reads the
scan buffer directly) -> PSUM DMA'd straight to DRAM.  Per-core output is
[O, T] without the output bias; the host transposes and adds bo.
"""

from contextlib import ExitStack

import numpy as np
import ml_dtypes

import concourse.bacc as bacc
import concourse.tile as tile
from concourse import mybir
from concourse.bass_utils import run_bass_kernel_spmd

BF16 = ml_dtypes.bfloat16
F32 = mybir.dt.float32
F32R = mybir.dt.float32r
BF = mybir.dt.bfloat16

B, T, D, H, O = 8, 16384, 256, 256, 256
L = 2048          # positions per chunk (power of 2)
NCHUNK = T // L   # 8 full chunks; position T (=16384) handled separately
SUB = 512         # matmul sub-chunk (one PSUM bank at f32)
LMAX = 11         # log2(L)

AluOp = mybir.AluOpType
ActFn = mybir.ActivationFunctionType


def _level_offsets():
    off = {1: 0}
    n = L // 2
    for lvl in range(1, LMAX):
        off[lvl + 1] = off[lvl] + n
        n //= 2
    return off, off[LMAX] + 1


LVL_OFF, LVL_TOTAL = _level_offsets()  # total = 2047


def build_nc():
    nc = bacc.Bacc()

    xt = nc.dram_tensor("xt", [D, T], BF, kind="ExternalInput")
    wzt = nc.dram_tensor("wzt", [D, H], BF, kind="ExternalInput")
    wht = nc.dram_tensor("wht", [D, H], BF, kind="ExternalInput")
    wot = nc.dram_tensor("wot", [H, O], F32R, kind="ExternalInput")
    bzp = nc.dram_tensor("bzp", [H, 1], F32, kind="ExternalInput")   # +bz
    bzn = nc.dram_tensor("bzn", [H, 1], F32, kind="ExternalInput")   # -bz
    bhb = nc.dram_tensor("bhb", [H, 1], F32, kind="ExternalInput")
    out = nc.dram_tensor("out", [O, T], F32, kind="ExternalOutput")

    with tile.TileContext(nc) as tc, ExitStack() as ctx:
        singles = ctx.enter_context(tc.tile_pool(name="singles", bufs=1))
        ab_pool = ctx.enter_context(tc.tile_pool(name="ab", bufs=2))
        lvl_pool = ctx.enter_context(tc.tile_pool(name="lvl", bufs=2))
        dbuf_pool = ctx.enter_context(tc.tile_pool(name="dbuf", bufs=2))
        tmp_pool = ctx.enter_context(tc.tile_pool(name="tmp", bufs=1))
        z_pool = ctx.enter_context(tc.tile_pool(name="zp", bufs=2))
        x_pool = ctx.enter_context(tc.tile_pool(name="xp", bufs=2))
        osb_pool = ctx.enter_context(tc.tile_pool(name="osb", bufs=2))
        psum_y = ctx.enter_context(tc.tile_pool(name="psy", bufs=2, space="PSUM"))
        psum_o = ctx.enter_context(tc.tile_pool(name="pso", bufs=2, space="PSUM"))

        # ---- constants ----
        wz_sb, wh_sb, wo_sb = [], [], []
        for k in range(2):
            wzk = singles.tile([128, H], BF, name=f"wzk{k}")
            nc.sync.dma_start(out=wzk, in_=wzt[k * 128:(k + 1) * 128, :])
            wz_sb.append(wzk)
            whk = singles.tile([128, H], BF, name=f"whk{k}")
            nc.sync.dma_start(out=whk, in_=wht[k * 128:(k + 1) * 128, :])
            wh_sb.append(whk)
            wok = singles.tile([128, O], F32R, name=f"wok{k}")
            nc.sync.dma_start(out=wok, in_=wot[k * 128:(k + 1) * 128, :])
            wo_sb.append(wok)
        bzp_sb, bzn_sb, bh_sb = [], [], []
        for h in range(2):
            pz = singles.tile([128, 1], F32, name=f"bzp{h}")
            nc.sync.dma_start(out=pz, in_=bzp[h * 128:(h + 1) * 128, :])
            bzp_sb.append(pz)
            nz = singles.tile([128, 1], F32, name=f"bzn{h}")
            nc.sync.dma_start(out=nz, in_=bzn[h * 128:(h + 1) * 128, :])
            bzn_sb.append(nz)
            hb = singles.tile([128, 1], F32, name=f"bh{h}")
            nc.sync.dma_start(out=hb, in_=bhb[h * 128:(h + 1) * 128, :])
            bh_sb.append(hb)

        # top-level bookkeeping, both halves fused: [128, 2, n]
        tops_A = singles.tile([128, 2, 8], F32, name="topsA")
        tops_B = singles.tile([128, 2, 8], F32, name="topsB")
        # spine: 0-3 sB12_0..3, 4 sA12_1, 5 sA12_2, 6 sA12_3,
        #        7 sB13_0, 8 sB13_1, 9 sA13_1, 10 sB14
        spine = singles.tile([128, 2, 12], F32, name="spine")
        otb = singles.tile([128, 2, 8], F32, name="otb")

        def top_combine(dstB, lB, rA, rB):
            """combine on [128,2,1] APs (gpsimd): dstB = lB*rA + rB."""
            t = tmp_pool.tile([128, 2, 1], F32, name="ttop", tag="ttop")
            nc.gpsimd.tensor_tensor(t, lB, rA, op=AluOp.mult)
            nc.gpsimd.tensor_tensor(dstB, t, rB, op=AluOp.add)

        def emit_subchunk_mats(x0, ncols, acol):
            """DMA x cols [x0, x0+ncols); z/h matmuls; sigmoids; b into
            a_buf/b_buf[:, :, acol:acol+ncols)."""
            xk = x_pool.tile([128, 2, SUB], BF, name="xk", tag="xk")
            nc.sync.dma_start(
                out=xk[:, :, :ncols],
                in_=xt[:, x0:x0 + ncols].rearrange("(k p) n -> p k n", p=128))
            for h in range(2):
                yz = psum_y.tile([128, SUB], F32, name="yz", tag=f"y{h}")
                for k in range(2):
                    nc.tensor.matmul(yz[:, :ncols],
                                     wz_sb[k][:, h * 128:(h + 1) * 128],
                                     xk[:, k, :ncols],
                                     start=(k == 0), stop=(k == 1))
                zt = z_pool.tile([128, SUB], F32, name="zt", tag=f"zt{h}")
                nc.scalar.activation(zt[:, :ncols], yz[:, :ncols], ActFn.Sigmoid,
                                     bias=bzp_sb[h][:, 0:1], scale=1.0)
                nc.scalar.activation(a_buf[:, h, acol:acol + ncols],
                                     yz[:, :ncols], ActFn.Sigmoid,
                                     bias=bzn_sb[h][:, 0:1], scale=-1.0)
                yh = psum_y.tile([128, SUB], F32, name="yh", tag=f"y{h}")
                for k in range(2):
                    nc.tensor.matmul(yh[:, :ncols],
                                     wh_sb[k][:, h * 128:(h + 1) * 128],
                                     xk[:, k, :ncols],
                                     start=(k == 0), stop=(k == 1))
                nc.vector.scalar_tensor_tensor(
                    b_buf[:, h, acol:acol + ncols], yh[:, :ncols],
                    bh_sb[h][:, 0:1], zt[:, :ncols],
                    op0=AluOp.add, op1=AluOp.mult)

        for c in range(NCHUNK):
            a_buf = ab_pool.tile([128, 2, L], F32, name="a_buf", tag="a")
            b_buf = ab_pool.tile([128, 2, L], F32, name="b_buf", tag="b")

            # ---- phase 1: matmuls + sigmoid -> a/b ----
            if c == 0:
                nc.vector.memset(a_buf[:, :, 0:1], 1.0)
                nc.vector.memset(b_buf[:, :, 0:1], 0.0)
                for s in range(4):
                    ncols = SUB if s < 3 else SUB - 1
                    emit_subchunk_mats(s * SUB, ncols, s * SUB + 1)
            else:
                base = c * L - 1
                for s in range(4):
                    emit_subchunk_mats(base + s * SUB, SUB, s * SUB)

            # ---- phase 2: up-sweep, entirely on gpsimd (its own chain,
            # overlaps the previous chunk's DVE down-sweep) ----
            Aup = lvl_pool.tile([128, 2, LVL_TOTAL], F32, name="Aup", tag="Au")
            Bup = lvl_pool.tile([128, 2, LVL_TOTAL], F32, name="Bup", tag="Bu")
            for lvl in range(LMAX):
                n = L >> lvl
                m = n // 2
                if lvl == 0:
                    sA, sB = a_buf, b_buf
                else:
                    o = LVL_OFF[lvl]
                    sA = Aup[:, :, o:o + n]
                    sB = Bup[:, :, o:o + n]
                o2 = LVL_OFF[lvl + 1]
                dA = Aup[:, :, o2:o2 + m]
                dB = Bup[:, :, o2:o2 + m]
                A_ev, A_od = sA[:, :, 0:n:2], sA[:, :, 1:n:2]
                B_ev, B_od = sB[:, :, 0:n:2], sB[:, :, 1:n:2]
                nc.gpsimd.tensor_tensor(dA, A_ev, B_ev, op=AluOp.mult)
                tu = tmp_pool.tile([128, 2, L // 2], F32, name="tu", tag="tu")
                nc.gpsimd.tensor_tensor(tu[:, :, :m], B_ev, A_od, op=AluOp.mult)
                nc.gpsimd.tensor_tensor(dB, tu[:, :, :m], B_od, op=AluOp.add)

            # ---- phase 3: top-level bookkeeping ----
            o11 = LVL_OFF[LMAX]
            EA = tops_A[:, :, c:c + 1]
            EB = tops_B[:, :, c:c + 1]
            nc.gpsimd.tensor_copy(EA, Aup[:, :, o11:o11 + 1])
            nc.gpsimd.tensor_copy(EB, Bup[:, :, o11:o11 + 1])
            sp = spine
            cc = lambda i: (tops_A[:, :, i:i + 1], tops_B[:, :, i:i + 1])
            if c == 0:
                nc.gpsimd.tensor_copy(otb[:, :, 0:1], EB)
            elif c == 1:
                top_combine(sp[:, :, 0:1], cc(0)[1], *cc(1))
                nc.gpsimd.tensor_copy(otb[:, :, 1:2], sp[:, :, 0:1])
            elif c == 2:
                top_combine(otb[:, :, 2:3], otb[:, :, 1:2], EA, EB)
            elif c == 3:
                top_combine(sp[:, :, 1:2], cc(2)[1], *cc(3))
                nc.gpsimd.tensor_tensor(sp[:, :, 4:5], cc(2)[0], cc(2)[1],
                                        op=AluOp.mult)          # sA12_1
                top_combine(sp[:, :, 7:8], sp[:, :, 0:1],
                            sp[:, :, 4:5], sp[:, :, 1:2])       # sB13_0
                nc.gpsimd.tensor_copy(otb[:, :, 3:4], sp[:, :, 7:8])
            elif c == 4:
                top_combine(otb[:, :, 4:5], otb[:, :, 3:4], EA, EB)
            elif c == 5:
                top_combine(sp[:, :, 2:3], cc(4)[1], *cc(5))    # sB12_2
                nc.gpsimd.tensor_tensor(sp[:, :, 5:6], cc(4)[0], cc(4)[1],
                                        op=AluOp.mult)          # sA12_2
                top_combine(otb[:, :, 5:6], otb[:, :, 3:4],
                            sp[:, :, 5:6], sp[:, :, 2:3])
            elif c == 6:
                top_combine(otb[:, :, 6:7], otb[:, :, 5:6], EA, EB)
            elif c == 7:
                top_combine(sp[:, :, 3:4], cc(6)[1], *cc(7))    # sB12_3
                nc.gpsimd.tensor_tensor(sp[:, :, 6:7], cc(6)[0], cc(6)[1],
                                        op=AluOp.mult)          # sA12_3
                top_combine(sp[:, :, 8:9], sp[:, :, 2:3],
                            sp[:, :, 6:7], sp[:, :, 3:4])       # sB13_1
                nc.gpsimd.tensor_tensor(sp[:, :, 9:10], sp[:, :, 5:6],
                                        sp[:, :, 2:3], op=AluOp.mult)  # sA13_1
                top_combine(sp[:, :, 10:11], sp[:, :, 7:8],
                            sp[:, :, 9:10], sp[:, :, 8:9])      # sB14
                nc.gpsimd.tensor_copy(otb[:, :, 7:8], sp[:, :, 10:11])

            # ---- phase 4: down-sweep into f32r dbuf ----
            dbuf = dbuf_pool.tile([128, 2, L + 1], F32R, name="dbuf", tag="d")
            if c == 0:
                nc.vector.memset(dbuf[:, :, 0:1].bitcast(F32), 0.0)
            else:
                nc.vector.tensor_copy(dbuf[:, :, 0:1], otb[:, :, c - 1:c])
            nc.vector.tensor_copy(dbuf[:, :, L:L + 1], otb[:, :, c:c + 1])
            for lvl in range(LMAX - 1, -1, -1):
                n = L >> lvl
                cnt = n // 2
                step = 1 << (lvl + 1)
                if lvl == 0:
                    A_src, B_src = a_buf, b_buf
                else:
                    o = LVL_OFF[lvl]
                    A_src = Aup[:, :, o:o + n]
                    B_src = Bup[:, :, o:o + n]
                A_ev = A_src[:, :, 0:n:2]
                B_ev = B_src[:, :, 0:n:2]
                Lh = dbuf[:, :, 0:L:step]
                Wt = dbuf[:, :, (1 << lvl):L:step]
                td = tmp_pool.tile([128, 2, L // 2], F32, name="td", tag="td")
                nc.vector.tensor_tensor(td[:, :, :cnt], Lh, A_ev, op=AluOp.mult)
                nc.vector.tensor_tensor(Wt, td[:, :, :cnt], B_ev, op=AluOp.add)

            # ---- phase 5: output matmul straight from dbuf (f32r) ----
            # Always process dbuf cols [1, 2049) = L cols (even-N subchunks,
            # required by f32r matmul).  For chunk 0 the first col is the
            # dummy position-0 value; skip it when storing.
            obase = c * L - 1
            for s in range(4):
                col0 = s * SUB
                po = psum_o.tile([128, 2, SUB], F32, name="po", tag="po")
                for oh in range(2):
                    for k in range(2):
                        nc.tensor.matmul(po[:, oh, :],
                                         wo_sb[k][:, oh * 128:(oh + 1) * 128],
                                         dbuf[:, k, 1 + col0:1 + col0 + SUB],
                                         start=(k == 0), stop=(k == 1))
                osb = osb_pool.tile([128, 2, SUB], F32, name="osb", tag="osb")
                nc.scalar.copy(osb, po)
                skip = 1 if (c == 0 and s == 0) else 0
                dst = out[:, obase + col0 + skip:obase + col0 + SUB]
                nc.sync.dma_start(
                    out=dst.rearrange("(two p) n -> p two n", p=128),
                    in_=osb[:, :, skip:])

            if c == NCHUNK - 1:
                last_dbuf = dbuf

        # ---- final position T: out[p] = out[p-1]*a + b ----
        xl = singles.tile([128, 2, 1], BF, name="xl")
        nc.sync.dma_start(out=xl,
                          in_=xt[:, T - 1:T].rearrange("(k p) n -> p k n", p=128))
        al = singles.tile([128, 2, 1], F32, name="al")
        bl = singles.tile([128, 2, 1], F32, name="bl")
        for h in range(2):
            yzl = psum_y.tile([128, SUB], F32, name="yzl", tag=f"y{h}")[:, 0:1]
            for k in range(2):
                nc.tensor.matmul(yzl, wz_sb[k][:, h * 128:(h + 1) * 128],
                                 xl[:, k, :], start=(k == 0), stop=(k == 1))
            zl = singles.tile([128, 1], F32, name=f"zl{h}")
            nc.scalar.activation(zl, yzl, ActFn.Sigmoid,
                                 bias=bzp_sb[h][:, 0:1], scale=1.0)
            nc.scalar.activation(al[:, h, :], yzl, ActFn.Sigmoid,
                                 bias=bzn_sb[h][:, 0:1], scale=-1.0)
            yhl = psum_y.tile([128, SUB], F32, name="yhl", tag=f"y{h}")[:, 0:1]
            for k in range(2):
                nc.tensor.matmul(yhl, wh_sb[k][:, h * 128:(h + 1) * 128],
                                 xl[:, k, :], start=(k == 0), stop=(k == 1))
            nc.vector.scalar_tensor_tensor(bl[:, h, :], yhl, bh_sb[h][:, 0:1],
                                           zl, op0=AluOp.add, op1=AluOp.mult)
        # f32r matmul needs even N: pad the single final column to 2.
        dl = singles.tile([128, 2, 2], F32R, name="dl")
        tl = singles.tile([128, 2, 1], F32, name="tl")
        nc.vector.memset(dl.bitcast(F32), 0.0)
        nc.vector.tensor_tensor(tl, last_dbuf[:, :, L:L + 1], al, op=AluOp.mult)
        nc.vector.tensor_tensor(dl[:, :, 0:1], tl, bl, op=AluOp.add)
        pol = psum_o.tile([128, 2, SUB], F32, name="pol", tag="po")[:, :, 0:2]
        for oh in range(2):
            for k in range(2):
                nc.tensor.matmul(pol[:, oh, :],
                                 wo_sb[k][:, oh * 128:(oh + 1) * 128],
                                 dl[:, k, :], start=(k == 0), stop=(k == 1))
        osl = singles.tile([128, 2, 1], F32, name="osl")
        nc.scalar.copy(osl, pol[:, :, 0:1])
        nc.sync.dma_start(
            out=out[:, T - 1:T].rearrange("(two p) n -> p two n", p=128),
            in_=osl)

    nc.compile()
    return nc


_NC_CACHE = {}


def _get_nc():
    if "nc" not in _NC_CACHE:
        _NC_CACHE["nc"] = build_nc()
    return _NC_CACHE["nc"]


def _prepare_in_maps(xs, Wz, bz, Wh, bh, Wo, bo):
    xs = np.asarray(xs, np.float32)
    Wz = np.asarray(Wz, np.float32)
    bz = np.asarray(bz, np.float32)
    Wh = np.asarray(Wh, np.float32)
    bh = np.asarray(bh, np.float32)
    Wo = np.asarray(Wo, np.float32)

    wzt = np.ascontiguousarray(Wz.T).astype(BF16)
    wht = np.ascontiguousarray(Wh.T).astype(BF16)
    wot = np.ascontiguousarray(Wo.T)          # f32 bits, f32r on device
    bzp = np.ascontiguousarray(bz.reshape(H, 1))
    bzn = np.ascontiguousarray((-bz).reshape(H, 1))
    bhb = np.ascontiguousarray(bh.reshape(H, 1))

    in_maps = []
    for i in range(B):
        xti = np.ascontiguousarray(xs[i].T).astype(BF16)
        in_maps.append({
            "xt": xti, "wzt": wzt, "wht": wht, "wot": wot,
            "bzp": bzp, "bzn": bzn, "bhb": bhb,
        })
    return in_maps


def _assemble(res, bo):
    bo = np.asarray(bo, np.float32)
    return np.stack([np.asarray(res.results[i]["out"], np.float32).T + bo
                     for i in range(B)], axis=0)


def run_traced(xs, Wz, bz, Wh, bh, Wo, bo, trace=True):
    in_maps = _prepare_in_maps(xs, Wz, bz, Wh, bh, Wo, bo)
    res = run_bass_kernel_spmd(_get_nc(), in_maps, core_ids=list(range(B)),
                               trace=trace)
    return _assemble(res, bo), res


def kernel(xs, Wz, bz, Wh, bh, Wo, bo):
    in_maps = _prepare_in_maps(xs, Wz, bz, Wh, bh, Wo, bo)
    res = run_bass_kernel_spmd(_get_nc(), in_maps, core_ids=list(range(B)))
    return _assemble(res, bo)


# revision 24
# speedup vs baseline: 1.2989x; 1.2989x over previous
"""MinGRU layer Trainium2 kernel.

Reference semantics (B=8, T=16384, D=H=O=256):
    zs = sigmoid(xs @ Wz.T + bz);  hs = xs @ Wh.T + bh
    a = concat([1], 1-zs);  b = concat([0], zs*hs)         (T+1 positions)
    states = jax.lax.associative_scan(combine, (a, b))[1][:, 1:]
    out = states @ Wo.T + bo
with combine((a0,b0),(a1,b1)) = (a0*b0, b0*a1 + b1).

The combine is NOT associative, so the result is defined by jax's exact
odd/even recursion tree.  We replicate that tree exactly:
  - positions split into 8 aligned chunks of L=2048 (+1 trailing position);
  - per-chunk bottom-up reduce ("up-sweep", keeping all tree levels);
  - a tiny cross-chunk scan over the 8 chunk-top elements following the same
    recursion (chunk prefixes + chunk-end outputs);
  - per-chunk top-down "down-sweep" filling every position's scan value.

Sharding: batch b=8 across the 8 cores (one sequence per core); weights
replicated.  The host pre-transposes/casts x and the weights.

Engine layout: PE does the three matmuls (bf16); ACT does sigmoids, PSUM
evacuation and the f32->bf16 state cast; the whole scan tree runs on DVE
(GpSimd shares SBUF ports with DVE and would serialize against it).  Both
hidden halves are fused per op via [128, 2, n] APs.  Emission is software-
pipelined: phase-1 of chunk c+2 is emitted between chunk c's tree and chunk
c+1's tree so its matmul/sigmoid/b-ops interleave into the DVE queue ahead
of the next tree chain, keeping PE/ACT busy while DVE walks the tree.
Per-core output is [O, T] without the output bias; the host transposes and
adds bo.
"""

from contextlib import ExitStack

import numpy as np
import ml_dtypes

import concourse.bacc as bacc
import concourse.tile as tile
from concourse import mybir
from concourse.bass_utils import run_bass_kernel_spmd

BF16 = ml_dtypes.bfloat16
F32 = mybir.dt.float32
BF = mybir.dt.bfloat16

B, T, D, H, O = 8, 16384, 256, 256, 256
L = 2048          # positions per chunk (power of 2)
NCHUNK = T // L   # 8 full chunks; position T (=16384) handled separately
SUB = 512         # matmul sub-chunk (one PSUM bank at f32)
LMAX = 11         # log2(L)

AluOp = mybir.AluOpType
ActFn = mybir.ActivationFunctionType


def _level_offsets():
    off = {1: 0}
    n = L // 2
    for lvl in range(1, LMAX):
        off[lvl + 1] = off[lvl] + n
        n //= 2
    return off, off[LMAX] + 1


LVL_OFF, LVL_TOTAL = _level_offsets()  # total = 2047


def build_nc():
    nc = bacc.Bacc()

    xt = nc.dram_tensor("xt", [D, T], BF, kind="ExternalInput")
    wzt = nc.dram_tensor("wzt", [D, H], BF, kind="ExternalInput")
    wht = nc.dram_tensor("wht", [D, H], BF, kind="ExternalInput")
    wot = nc.dram_tensor("wot", [H, O], BF, kind="ExternalInput")
    bzp = nc.dram_tensor("bzp", [H, 1], F32, kind="ExternalInput")   # +bz
    bzn = nc.dram_tensor("bzn", [H, 1], F32, kind="ExternalInput")   # -bz
    bhb = nc.dram_tensor("bhb", [H, 1], F32, kind="ExternalInput")
    out = nc.dram_tensor("out", [O, T], F32, kind="ExternalOutput")

    with tile.TileContext(nc) as tc, ExitStack() as ctx:
        singles = ctx.enter_context(tc.tile_pool(name="singles", bufs=1))
        ab_pool = ctx.enter_context(tc.tile_pool(name="ab", bufs=3))
        lvl_pool = ctx.enter_context(tc.tile_pool(name="lvl", bufs=1))
        dbuf_pool = ctx.enter_context(tc.tile_pool(name="dbuf", bufs=1))
        st_pool = ctx.enter_context(tc.tile_pool(name="st", bufs=2))
        tmp_pool = ctx.enter_context(tc.tile_pool(name="tmp", bufs=1))
        z_pool = ctx.enter_context(tc.tile_pool(name="zp", bufs=4))
        x_pool = ctx.enter_context(tc.tile_pool(name="xp", bufs=2))
        osb_pool = ctx.enter_context(tc.tile_pool(name="osb", bufs=2))
        psum_y = ctx.enter_context(tc.tile_pool(name="psy", bufs=3, space="PSUM"))
        psum_o = ctx.enter_context(tc.tile_pool(name="pso", bufs=1, space="PSUM"))

        # ---- constants ----
        wz_sb, wh_sb, wo_sb = [], [], []
        for k in range(2):
            wzk = singles.tile([128, H], BF, name=f"wzk{k}")
            nc.sync.dma_start(out=wzk, in_=wzt[k * 128:(k + 1) * 128, :])
            wz_sb.append(wzk)
            whk = singles.tile([128, H], BF, name=f"whk{k}")
            nc.sync.dma_start(out=whk, in_=wht[k * 128:(k + 1) * 128, :])
            wh_sb.append(whk)
            wok = singles.tile([128, O], BF, name=f"wok{k}")
            nc.sync.dma_start(out=wok, in_=wot[k * 128:(k + 1) * 128, :])
            wo_sb.append(wok)
        bzp_sb, bzn_sb, bh_sb = [], [], []
        for h in range(2):
            pz = singles.tile([128, 1], F32, name=f"bzp{h}")
            nc.sync.dma_start(out=pz, in_=bzp[h * 128:(h + 1) * 128, :])
            bzp_sb.append(pz)
            nz = singles.tile([128, 1], F32, name=f"bzn{h}")
            nc.sync.dma_start(out=nz, in_=bzn[h * 128:(h + 1) * 128, :])
            bzn_sb.append(nz)
            hb = singles.tile([128, 1], F32, name=f"bh{h}")
            nc.sync.dma_start(out=hb, in_=bhb[h * 128:(h + 1) * 128, :])
            bh_sb.append(hb)

        # top-level bookkeeping, both halves fused: [128, 2, n]
        tops_A = singles.tile([128, 2, 8], F32, name="topsA")
        tops_B = singles.tile([128, 2, 8], F32, name="topsB")
        # spine: 0-3 sB12_0..3, 4 sA12_1, 5 sA12_2, 6 sA12_3,
        #        7 sB13_0, 8 sB13_1, 9 sA13_1, 10 sB14
        spine = singles.tile([128, 2, 12], F32, name="spine")
        otb = singles.tile([128, 2, 8], F32, name="otb")

        def top_combine(dstB, lB, rA, rB):
            """combine on [128,2,1] APs (DVE): dstB = lB*rA + rB."""
            t = tmp_pool.tile([128, 2, 1], F32, name="ttop", tag="ttop")
            nc.vector.tensor_tensor(t, lB, rA, op=AluOp.mult)
            nc.vector.tensor_tensor(dstB, t, rB, op=AluOp.add)

        abufs = {}

        def emit_phase1(c):
            """DMA/matmul/sigmoid/b for chunk c into fresh a/b tiles."""
            a_buf = ab_pool.tile([128, 2, L], F32, name="a_buf", tag="a")
            b_buf = ab_pool.tile([128, 2, L], F32, name="b_buf", tag="b")
            abufs[c] = (a_buf, b_buf)
            if c == 0:
                nc.vector.memset(a_buf[:, :, 0:1], 1.0)
                nc.vector.memset(b_buf[:, :, 0:1], 0.0)
                subs = [(s * SUB, SUB if s < 3 else SUB - 1, s * SUB + 1)
                        for s in range(4)]
            else:
                base = c * L - 1
                subs = [(base + s * SUB, SUB, s * SUB) for s in range(4)]
            for x0, ncols, acol in subs:
                xk = x_pool.tile([128, 2, SUB], BF, name="xk", tag="xk")
                nc.sync.dma_start(
                    out=xk[:, :, :ncols],
                    in_=xt[:, x0:x0 + ncols].rearrange("(k p) n -> p k n", p=128))
                for h in range(2):
                    yz = psum_y.tile([128, SUB], F32, name="yz", tag=f"y{h}")
                    for k in range(2):
                        nc.tensor.matmul(yz[:, :ncols],
                                         wz_sb[k][:, h * 128:(h + 1) * 128],
                                         xk[:, k, :ncols],
                                         start=(k == 0), stop=(k == 1))
                    zt = z_pool.tile([128, SUB], F32, name="zt", tag=f"zt{h}")
                    nc.scalar.activation(zt[:, :ncols], yz[:, :ncols],
                                         ActFn.Sigmoid,
                                         bias=bzp_sb[h][:, 0:1], scale=1.0)
                    nc.scalar.activation(a_buf[:, h, acol:acol + ncols],
                                         yz[:, :ncols], ActFn.Sigmoid,
                                         bias=bzn_sb[h][:, 0:1], scale=-1.0)
                    yh = psum_y.tile([128, SUB], F32, name="yh", tag=f"y{h}")
                    for k in range(2):
                        nc.tensor.matmul(yh[:, :ncols],
                                         wh_sb[k][:, h * 128:(h + 1) * 128],
                                         xk[:, k, :ncols],
                                         start=(k == 0), stop=(k == 1))
                    nc.vector.scalar_tensor_tensor(
                        b_buf[:, h, acol:acol + ncols], yh[:, :ncols],
                        bh_sb[h][:, 0:1], zt[:, :ncols],
                        op0=AluOp.add, op1=AluOp.mult)

        def emit_tree_and_out(c):
            a_buf, b_buf = abufs.pop(c)
            # ---- phase 2: up-sweep (DVE) ----
            Aup = lvl_pool.tile([128, 2, LVL_TOTAL], F32, name="Aup", tag="Au")
            Bup = lvl_pool.tile([128, 2, LVL_TOTAL], F32, name="Bup", tag="Bu")
            for lvl in range(LMAX):
                n = L >> lvl
                m = n // 2
                if lvl == 0:
                    sA, sB = a_buf, b_buf
                else:
                    o = LVL_OFF[lvl]
                    sA = Aup[:, :, o:o + n]
                    sB = Bup[:, :, o:o + n]
                o2 = LVL_OFF[lvl + 1]
                dA = Aup[:, :, o2:o2 + m]
                dB = Bup[:, :, o2:o2 + m]
                A_ev, A_od = sA[:, :, 0:n:2], sA[:, :, 1:n:2]
                B_ev, B_od = sB[:, :, 0:n:2], sB[:, :, 1:n:2]
                nc.vector.tensor_tensor(dA, A_ev, B_ev, op=AluOp.mult)
                tu = tmp_pool.tile([128, 2, L // 2], F32, name="tu", tag="tmp")
                nc.vector.tensor_tensor(tu[:, :, :m], B_ev, A_od, op=AluOp.mult)
                nc.vector.tensor_tensor(dB, tu[:, :, :m], B_od, op=AluOp.add)

            # ---- phase 3: top-level bookkeeping (DVE) ----
            o11 = LVL_OFF[LMAX]
            EA = tops_A[:, :, c:c + 1]
            EB = tops_B[:, :, c:c + 1]
            nc.vector.tensor_copy(EA, Aup[:, :, o11:o11 + 1])
            nc.vector.tensor_copy(EB, Bup[:, :, o11:o11 + 1])
            sp = spine
            cc = lambda i: (tops_A[:, :, i:i + 1], tops_B[:, :, i:i + 1])
            if c == 0:
                nc.vector.tensor_copy(otb[:, :, 0:1], EB)
            elif c == 1:
                top_combine(sp[:, :, 0:1], cc(0)[1], *cc(1))
                nc.vector.tensor_copy(otb[:, :, 1:2], sp[:, :, 0:1])
            elif c == 2:
                top_combine(otb[:, :, 2:3], otb[:, :, 1:2], EA, EB)
            elif c == 3:
                top_combine(sp[:, :, 1:2], cc(2)[1], *cc(3))
                nc.vector.tensor_tensor(sp[:, :, 4:5], cc(2)[0], cc(2)[1],
                                        op=AluOp.mult)          # sA12_1
                top_combine(sp[:, :, 7:8], sp[:, :, 0:1],
                            sp[:, :, 4:5], sp[:, :, 1:2])       # sB13_0
                nc.vector.tensor_copy(otb[:, :, 3:4], sp[:, :, 7:8])
            elif c == 4:
                top_combine(otb[:, :, 4:5], otb[:, :, 3:4], EA, EB)
            elif c == 5:
                top_combine(sp[:, :, 2:3], cc(4)[1], *cc(5))    # sB12_2
                nc.vector.tensor_tensor(sp[:, :, 5:6], cc(4)[0], cc(4)[1],
                                        op=AluOp.mult)          # sA12_2
                top_combine(otb[:, :, 5:6], otb[:, :, 3:4],
                            sp[:, :, 5:6], sp[:, :, 2:3])
            elif c == 6:
                top_combine(otb[:, :, 6:7], otb[:, :, 5:6], EA, EB)
            elif c == 7:
                top_combine(sp[:, :, 3:4], cc(6)[1], *cc(7))    # sB12_3
                nc.vector.tensor_tensor(sp[:, :, 6:7], cc(6)[0], cc(6)[1],
                                        op=AluOp.mult)          # sA12_3
                top_combine(sp[:, :, 8:9], sp[:, :, 2:3],
                            sp[:, :, 6:7], sp[:, :, 3:4])       # sB13_1
                nc.vector.tensor_tensor(sp[:, :, 9:10], sp[:, :, 5:6],
                                        sp[:, :, 2:3], op=AluOp.mult)  # sA13_1
                top_combine(sp[:, :, 10:11], sp[:, :, 7:8],
                            sp[:, :, 9:10], sp[:, :, 8:9])      # sB14
                nc.vector.tensor_copy(otb[:, :, 7:8], sp[:, :, 10:11])

            # ---- phase 4: down-sweep (DVE) ----
            dbuf = dbuf_pool.tile([128, 2, L + 1], F32, name="dbuf", tag="d")
            if c == 0:
                nc.vector.memset(dbuf[:, :, 0:1], 0.0)
            else:
                nc.vector.tensor_copy(dbuf[:, :, 0:1], otb[:, :, c - 1:c])
            nc.vector.tensor_copy(dbuf[:, :, L:L + 1], otb[:, :, c:c + 1])
            for lvl in range(LMAX - 1, -1, -1):
                n = L >> lvl
                cnt = n // 2
                step = 1 << (lvl + 1)
                if lvl == 0:
                    A_src, B_src = a_buf, b_buf
                else:
                    o = LVL_OFF[lvl]
                    A_src = Aup[:, :, o:o + n]
                    B_src = Bup[:, :, o:o + n]
                A_ev = A_src[:, :, 0:n:2]
                B_ev = B_src[:, :, 0:n:2]
                Lh = dbuf[:, :, 0:L:step]
                Wt = dbuf[:, :, (1 << lvl):L:step]
                td = tmp_pool.tile([128, 2, L // 2], F32, name="td", tag="tmp")
                nc.vector.tensor_tensor(td[:, :, :cnt], Lh, A_ev, op=AluOp.mult)
                nc.vector.tensor_tensor(Wt, td[:, :, :cnt], B_ev, op=AluOp.add)

            # ---- phase 5: cast + output matmul + store ----
            # states live in dbuf cols [1, 2049); chunk 0's col 1 is the dummy
            # position-0 value, skipped at DMA time.
            obase = c * L - 1
            st = st_pool.tile([128, 2, L], BF, name="st", tag="st")
            for s in range(4):
                col0 = s * SUB
                nc.scalar.copy(st[:, :, col0:col0 + SUB],
                               dbuf[:, :, 1 + col0:1 + col0 + SUB])
                po = psum_o.tile([128, 2, SUB], F32, name="po", tag="po")
                for oh in range(2):
                    for k in range(2):
                        nc.tensor.matmul(po[:, oh, :],
                                         wo_sb[k][:, oh * 128:(oh + 1) * 128],
                                         st[:, k, col0:col0 + SUB],
                                         start=(k == 0), stop=(k == 1))
                osb = osb_pool.tile([128, 2, SUB], F32, name="osb", tag="osb")
                nc.scalar.copy(osb, po)
                skip = 1 if (c == 0 and s == 0) else 0
                dst = out[:, obase + col0 + skip:obase + col0 + SUB]
                nc.sync.dma_start(
                    out=dst.rearrange("(two p) n -> p two n", p=128),
                    in_=osb[:, :, skip:])
            return dbuf

        # ---- software-pipelined emission ----
        emit_phase1(0)
        emit_phase1(1)
        for c in range(NCHUNK):
            last_dbuf = emit_tree_and_out(c)
            if c + 2 < NCHUNK:
                emit_phase1(c + 2)

        # ---- final position T: out[p] = out[p-1]*a + b ----
        xl = singles.tile([128, 2, 1], BF, name="xl")
        nc.sync.dma_start(out=xl,
                          in_=xt[:, T - 1:T].rearrange("(k p) n -> p k n", p=128))
        al = singles.tile([128, 2, 1], F32, name="al")
        bl = singles.tile([128, 2, 1], F32, name="bl")
        for h in range(2):
            yzl = psum_y.tile([128, SUB], F32, name="yzl", tag=f"y{h}")[:, 0:1]
            for k in range(2):
                nc.tensor.matmul(yzl, wz_sb[k][:, h * 128:(h + 1) * 128],
                                 xl[:, k, :], start=(k == 0), stop=(k == 1))
            zl = singles.tile([128, 1], F32, name=f"zl{h}")
            nc.scalar.activation(zl, yzl, ActFn.Sigmoid,
                                 bias=bzp_sb[h][:, 0:1], scale=1.0)
            nc.scalar.activation(al[:, h, :], yzl, ActFn.Sigmoid,
                                 bias=bzn_sb[h][:, 0:1], scale=-1.0)
            yhl = psum_y.tile([128, SUB], F32, name="yhl", tag=f"y{h}")[:, 0:1]
            for k in range(2):
                nc.tensor.matmul(yhl, wh_sb[k][:, h * 128:(h + 1) * 128],
                                 xl[:, k, :], start=(k == 0), stop=(k == 1))
            nc.vector.scalar_tensor_tensor(bl[:, h, :], yhl, bh_sb[h][:, 0:1],
                                           zl, op0=AluOp.add, op1=AluOp.mult)
        dl = singles.tile([128, 2, 1], F32, name="dl")
        sl = singles.tile([128, 2, 1], BF, name="sl")
        nc.vector.tensor_tensor(dl, last_dbuf[:, :, L:L + 1], al, op=AluOp.mult)
        nc.vector.tensor_tensor(dl, dl, bl, op=AluOp.add)
        nc.scalar.copy(sl, dl)
        pol = psum_o.tile([128, 2, SUB], F32, name="pol", tag="po")[:, :, 0:1]
        for oh in range(2):
            for k in range(2):
                nc.tensor.matmul(pol[:, oh, :],
                                 wo_sb[k][:, oh * 128:(oh + 1) * 128],
                                 sl[:, k, :], start=(k == 0), stop=(k == 1))
        osl = singles.tile([128, 2, 1], F32, name="osl")
        nc.scalar.copy(osl, pol)
        nc.sync.dma_start(
            out=out[:, T - 1:T].rearrange("(two p) n -> p two n", p=128),
            in_=osl)

    nc.compile()
    return nc


_NC_CACHE = {}


def _get_nc():
    if "nc" not in _NC_CACHE:
        _NC_CACHE["nc"] = build_nc()
    return _NC_CACHE["nc"]


def _prepare_in_maps(xs, Wz, bz, Wh, bh, Wo, bo):
    xs = np.asarray(xs, np.float32)
    Wz = np.asarray(Wz, np.float32)
    bz = np.asarray(bz, np.float32)
    Wh = np.asarray(Wh, np.float32)
    bh = np.asarray(bh, np.float32)
    Wo = np.asarray(Wo, np.float32)

    wzt = np.ascontiguousarray(Wz.T).astype(BF16)
    wht = np.ascontiguousarray(Wh.T).astype(BF16)
    wot = np.ascontiguousarray(Wo.T).astype(BF16)
    bzp = np.ascontiguousarray(bz.reshape(H, 1))
    bzn = np.ascontiguousarray((-bz).reshape(H, 1))
    bhb = np.ascontiguousarray(bh.reshape(H, 1))

    in_maps = []
    for i in range(B):
        xti = np.ascontiguousarray(xs[i].T).astype(BF16)
        in_maps.append({
            "xt": xti, "wzt": wzt, "wht": wht, "wot": wot,
            "bzp": bzp, "bzn": bzn, "bhb": bhb,
        })
    return in_maps


def _assemble(res, bo):
    bo = np.asarray(bo, np.float32)
    return np.stack([np.asarray(res.results[i]["out"], np.float32).T + bo
                     for i in range(B)], axis=0)


def run_traced(xs, Wz, bz, Wh, bh, Wo, bo, trace=True):
    in_maps = _prepare_in_maps(xs, Wz, bz, Wh, bh, Wo, bo)
    res = run_bass_kernel_spmd(_get_nc(), in_maps, core_ids=list(range(B)),
                               trace=trace)
    return _assemble(res, bo), res


def kernel(xs, Wz, bz, Wh, bh, Wo, bo):
    in_maps = _prepare_in_maps(xs, Wz, bz, Wh, bh, Wo, bo)
    res = run_bass_kernel_spmd(_get_nc(), in_maps, core_ids=list(range(B)))
    return _assemble(res, bo)


# revision 25
# speedup vs baseline: 1.3053x; 1.0049x over previous
"""MinGRU layer Trainium2 kernel.

Reference semantics (B=8, T=16384, D=H=O=256):
    zs = sigmoid(xs @ Wz.T + bz);  hs = xs @ Wh.T + bh
    a = concat([1], 1-zs);  b = concat([0], zs*hs)         (T+1 positions)
    states = jax.lax.associative_scan(combine, (a, b))[1][:, 1:]
    out = states @ Wo.T + bo
with combine((a0,b0),(a1,b1)) = (a0*b0, b0*a1 + b1).

The combine is NOT associative, so the result is defined by jax's exact
odd/even recursion tree.  We replicate that tree exactly:
  - positions split into 8 aligned chunks of L=2048 (+1 trailing position);
  - per-chunk bottom-up reduce ("up-sweep", keeping all tree levels);
  - a tiny cross-chunk scan over the 8 chunk-top elements following the same
    recursion (chunk prefixes + chunk-end outputs);
  - per-chunk top-down "down-sweep" filling every position's scan value.

Sharding: batch b=8 across the 8 cores (one sequence per core); weights
replicated.  The host pre-transposes/casts x and the weights.

Engine layout: PE does the three matmuls (bf16); ACT does sigmoids, PSUM
evacuation and the f32->bf16 state cast; the whole scan tree runs on DVE
(GpSimd shares SBUF ports with DVE and would serialize against it).  Both
hidden halves are fused per op via [128, 2, n] APs.  Emission is software-
pipelined: phase-1 of chunk c+2 is emitted between chunk c's tree and chunk
c+1's tree so its matmul/sigmoid/b-ops interleave into the DVE queue ahead
of the next tree chain, keeping PE/ACT busy while DVE walks the tree.
Per-core output is [O, T] without the output bias; the host transposes and
adds bo.
"""

from contextlib import ExitStack

import numpy as np
import ml_dtypes

import concourse.bacc as bacc
import concourse.tile as tile
from concourse import mybir
from concourse.bass_utils import run_bass_kernel_spmd

BF16 = ml_dtypes.bfloat16
F32 = mybir.dt.float32
BF = mybir.dt.bfloat16

B, T, D, H, O = 8, 16384, 256, 256, 256
L = 2048          # positions per chunk (power of 2)
NCHUNK = T // L   # 8 full chunks; position T (=16384) handled separately
SUB = 512         # matmul sub-chunk (one PSUM bank at f32)
LMAX = 11         # log2(L)

AluOp = mybir.AluOpType
ActFn = mybir.ActivationFunctionType


def _level_offsets():
    off = {1: 0}
    n = L // 2
    for lvl in range(1, LMAX):
        off[lvl + 1] = off[lvl] + n
        n //= 2
    return off, off[LMAX] + 1


LVL_OFF, LVL_TOTAL = _level_offsets()  # total = 2047


def build_nc():
    nc = bacc.Bacc()

    xt = nc.dram_tensor("xt", [D, T], BF, kind="ExternalInput")
    wzt = nc.dram_tensor("wzt", [D, H], BF, kind="ExternalInput")
    wht = nc.dram_tensor("wht", [D, H], BF, kind="ExternalInput")
    wot = nc.dram_tensor("wot", [H, O], BF, kind="ExternalInput")
    bzp = nc.dram_tensor("bzp", [H, 1], F32, kind="ExternalInput")   # +bz
    bzn = nc.dram_tensor("bzn", [H, 1], F32, kind="ExternalInput")   # -bz
    bhb = nc.dram_tensor("bhb", [H, 1], F32, kind="ExternalInput")
    out = nc.dram_tensor("out", [O, T], F32, kind="ExternalOutput")

    with tile.TileContext(nc) as tc, ExitStack() as ctx:
        singles = ctx.enter_context(tc.tile_pool(name="singles", bufs=1))
        ab_pool = ctx.enter_context(tc.tile_pool(name="ab", bufs=3))
        lvl_pool = ctx.enter_context(tc.tile_pool(name="lvl", bufs=1))
        dbuf_pool = ctx.enter_context(tc.tile_pool(name="dbuf", bufs=1))
        st_pool = ctx.enter_context(tc.tile_pool(name="st", bufs=2))
        tmp_pool = ctx.enter_context(tc.tile_pool(name="tmp", bufs=1))
        z_pool = ctx.enter_context(tc.tile_pool(name="zp", bufs=4))
        x_pool = ctx.enter_context(tc.tile_pool(name="xp", bufs=2))
        osb_pool = ctx.enter_context(tc.tile_pool(name="osb", bufs=2))
        psum_y = ctx.enter_context(tc.tile_pool(name="psy", bufs=3, space="PSUM"))
        psum_o = ctx.enter_context(tc.tile_pool(name="pso", bufs=1, space="PSUM"))

        # ---- constants ----
        wz_sb, wh_sb, wo_sb = [], [], []
        for k in range(2):
            wzk = singles.tile([128, H], BF, name=f"wzk{k}")
            nc.sync.dma_start(out=wzk, in_=wzt[k * 128:(k + 1) * 128, :])
            wz_sb.append(wzk)
            whk = singles.tile([128, H], BF, name=f"whk{k}")
            nc.sync.dma_start(out=whk, in_=wht[k * 128:(k + 1) * 128, :])
            wh_sb.append(whk)
            wok = singles.tile([128, O], BF, name=f"wok{k}")
            nc.sync.dma_start(out=wok, in_=wot[k * 128:(k + 1) * 128, :])
            wo_sb.append(wok)
        bzp_sb, bzn_sb, bh_sb = [], [], []
        for h in range(2):
            pz = singles.tile([128, 1], F32, name=f"bzp{h}")
            nc.sync.dma_start(out=pz, in_=bzp[h * 128:(h + 1) * 128, :])
            bzp_sb.append(pz)
            nz = singles.tile([128, 1], F32, name=f"bzn{h}")
            nc.sync.dma_start(out=nz, in_=bzn[h * 128:(h + 1) * 128, :])
            bzn_sb.append(nz)
            hb = singles.tile([128, 1], F32, name=f"bh{h}")
            nc.sync.dma_start(out=hb, in_=bhb[h * 128:(h + 1) * 128, :])
            bh_sb.append(hb)

        # top-level bookkeeping, both halves fused: [128, 2, n]
        tops_A = singles.tile([128, 2, 8], F32, name="topsA")
        tops_B = singles.tile([128, 2, 8], F32, name="topsB")
        # spine: 0-3 sB12_0..3, 4 sA12_1, 5 sA12_2, 6 sA12_3,
        #        7 sB13_0, 8 sB13_1, 9 sA13_1, 10 sB14
        spine = singles.tile([128, 2, 12], F32, name="spine")
        otb = singles.tile([128, 2, 8], F32, name="otb")

        def top_combine(dstB, lB, rA, rB):
            """combine on [128,2,1] APs (DVE): dstB = lB*rA + rB."""
            t = tmp_pool.tile([128, 2, 1], F32, name="ttop", tag="ttop")
            nc.vector.tensor_tensor(t, lB, rA, op=AluOp.mult)
            nc.vector.tensor_tensor(dstB, t, rB, op=AluOp.add)

        abufs = {}

        def emit_phase1(c):
            """DMA/matmul/sigmoid/b for chunk c into fresh a/b tiles."""
            a_buf = ab_pool.tile([128, 2, L], F32, name="a_buf", tag="a")
            b_buf = ab_pool.tile([128, 2, L], F32, name="b_buf", tag="b")
            abufs[c] = (a_buf, b_buf)
            if c == 0:
                nc.vector.memset(a_buf[:, :, 0:1], 1.0)
                nc.vector.memset(b_buf[:, :, 0:1], 0.0)
                subs = [(s * SUB, SUB if s < 3 else SUB - 1, s * SUB + 1)
                        for s in range(4)]
            else:
                base = c * L - 1
                subs = [(base + s * SUB, SUB, s * SUB) for s in range(4)]
            for x0, ncols, acol in subs:
                xk = x_pool.tile([128, 2, SUB], BF, name="xk", tag="xk")
                nc.sync.dma_start(
                    out=xk[:, :, :ncols],
                    in_=xt[:, x0:x0 + ncols].rearrange("(k p) n -> p k n", p=128))
                for h in range(2):
                    yz = psum_y.tile([128, SUB], F32, name="yz", tag=f"y{h}")
                    for k in range(2):
                        nc.tensor.matmul(yz[:, :ncols],
                                         wz_sb[k][:, h * 128:(h + 1) * 128],
                                         xk[:, k, :ncols],
                                         start=(k == 0), stop=(k == 1))
                    zt = z_pool.tile([128, SUB], F32, name="zt", tag=f"zt{h}")
                    nc.scalar.activation(zt[:, :ncols], yz[:, :ncols],
                                         ActFn.Sigmoid,
                                         bias=bzp_sb[h][:, 0:1], scale=1.0)
                    nc.scalar.activation(a_buf[:, h, acol:acol + ncols],
                                         yz[:, :ncols], ActFn.Sigmoid,
                                         bias=bzn_sb[h][:, 0:1], scale=-1.0)
                    yh = psum_y.tile([128, SUB], F32, name="yh", tag=f"y{h}")
                    for k in range(2):
                        nc.tensor.matmul(yh[:, :ncols],
                                         wh_sb[k][:, h * 128:(h + 1) * 128],
                                         xk[:, k, :ncols],
                                         start=(k == 0), stop=(k == 1))
                    nc.vector.scalar_tensor_tensor(
                        b_buf[:, h, acol:acol + ncols], yh[:, :ncols],
                        bh_sb[h][:, 0:1], zt[:, :ncols],
                        op0=AluOp.add, op1=AluOp.mult)

        def emit_tree_and_out(c):
            a_buf, b_buf = abufs.pop(c)
            # ---- phase 2: up-sweep (DVE) ----
            Aup = lvl_pool.tile([128, 2, LVL_TOTAL], F32, name="Aup", tag="Au")
            Bup = lvl_pool.tile([128, 2, LVL_TOTAL], F32, name="Bup", tag="Bu")
            for lvl in range(LMAX):
                n = L >> lvl
                m = n // 2
                if lvl == 0:
                    sA, sB = a_buf, b_buf
                else:
                    o = LVL_OFF[lvl]
                    sA = Aup[:, :, o:o + n]
                    sB = Bup[:, :, o:o + n]
                o2 = LVL_OFF[lvl + 1]
                dA = Aup[:, :, o2:o2 + m]
                dB = Bup[:, :, o2:o2 + m]
                A_ev, A_od = sA[:, :, 0:n:2], sA[:, :, 1:n:2]
                B_ev, B_od = sB[:, :, 0:n:2], sB[:, :, 1:n:2]
                nc.vector.tensor_tensor(dA, A_ev, B_ev, op=AluOp.mult)
                tu = tmp_pool.tile([128, 2, L // 2], F32, name="tu", tag="tmp")
                nc.vector.tensor_tensor(tu[:, :, :m], B_ev, A_od, op=AluOp.mult)
                nc.vector.tensor_tensor(dB, tu[:, :, :m], B_od, op=AluOp.add)

            # ---- phase 3: top-level bookkeeping (DVE) ----
            o11 = LVL_OFF[LMAX]
            EA = tops_A[:, :, c:c + 1]
            EB = tops_B[:, :, c:c + 1]
            nc.vector.tensor_copy(EA, Aup[:, :, o11:o11 + 1])
            nc.vector.tensor_copy(EB, Bup[:, :, o11:o11 + 1])
            sp = spine
            cc = lambda i: (tops_A[:, :, i:i + 1], tops_B[:, :, i:i + 1])
            if c == 0:
                nc.vector.tensor_copy(otb[:, :, 0:1], EB)
            elif c == 1:
                top_combine(sp[:, :, 0:1], cc(0)[1], *cc(1))
                nc.vector.tensor_copy(otb[:, :, 1:2], sp[:, :, 0:1])
            elif c == 2:
                top_combine(otb[:, :, 2:3], otb[:, :, 1:2], EA, EB)
            elif c == 3:
                top_combine(sp[:, :, 1:2], cc(2)[1], *cc(3))
                nc.vector.tensor_tensor(sp[:, :, 4:5], cc(2)[0], cc(2)[1],
                                        op=AluOp.mult)          # sA12_1
                top_combine(sp[:, :, 7:8], sp[:, :, 0:1],
                            sp[:, :, 4:5], sp[:, :, 1:2])       # sB13_0
                nc.vector.tensor_copy(otb[:, :, 3:4], sp[:, :, 7:8])
            elif c == 4:
                top_combine(otb[:, :, 4:5], otb[:, :, 3:4], EA, EB)
            elif c == 5:
                top_combine(sp[:, :, 2:3], cc(4)[1], *cc(5))    # sB12_2
                nc.vector.tensor_tensor(sp[:, :, 5:6], cc(4)[0], cc(4)[1],
                                        op=AluOp.mult)          # sA12_2
                top_combine(otb[:, :, 5:6], otb[:, :, 3:4],
                            sp[:, :, 5:6], sp[:, :, 2:3])
            elif c == 6:
                top_combine(otb[:, :, 6:7], otb[:, :, 5:6], EA, EB)
            elif c == 7:
                top_combine(sp[:, :, 3:4], cc(6)[1], *cc(7))    # sB12_3
                nc.vector.tensor_tensor(sp[:, :, 6:7], cc(6)[0], cc(6)[1],
                                        op=AluOp.mult)          # sA12_3
                top_combine(sp[:, :, 8:9], sp[:, :, 2:3],
                            sp[:, :, 6:7], sp[:, :, 3:4])       # sB13_1
                nc.vector.tensor_tensor(sp[:, :, 9:10], sp[:, :, 5:6],
                                        sp[:, :, 2:3], op=AluOp.mult)  # sA13_1
                top_combine(sp[:, :, 10:11], sp[:, :, 7:8],
                            sp[:, :, 9:10], sp[:, :, 8:9])      # sB14
                nc.vector.tensor_copy(otb[:, :, 7:8], sp[:, :, 10:11])

            # ---- phase 4: down-sweep (DVE) ----
            dbuf = dbuf_pool.tile([128, 2, L + 1], F32, name="dbuf", tag="d")
            if c == 0:
                nc.vector.memset(dbuf[:, :, 0:1], 0.0)
            else:
                nc.vector.tensor_copy(dbuf[:, :, 0:1], otb[:, :, c - 1:c])
            nc.vector.tensor_copy(dbuf[:, :, L:L + 1], otb[:, :, c:c + 1])
            for lvl in range(LMAX - 1, -1, -1):
                n = L >> lvl
                cnt = n // 2
                step = 1 << (lvl + 1)
                if lvl == 0:
                    A_src, B_src = a_buf, b_buf
                else:
                    o = LVL_OFF[lvl]
                    A_src = Aup[:, :, o:o + n]
                    B_src = Bup[:, :, o:o + n]
                A_ev = A_src[:, :, 0:n:2]
                B_ev = B_src[:, :, 0:n:2]
                Lh = dbuf[:, :, 0:L:step]
                Wt = dbuf[:, :, (1 << lvl):L:step]
                td = tmp_pool.tile([128, 2, L // 2], F32, name="td", tag="tmp")
                nc.vector.tensor_tensor(td[:, :, :cnt], Lh, A_ev, op=AluOp.mult)
                nc.vector.tensor_tensor(Wt, td[:, :, :cnt], B_ev, op=AluOp.add)

            return a_buf, b_buf, Aup, Bup, dbuf

        def emit_out(c, dbuf):
            # ---- phase 5: cast + output matmul + store ----
            # states live in dbuf cols [1, 2049); chunk 0's col 1 is the dummy
            # position-0 value, skipped at DMA time.
            obase = c * L - 1
            st = st_pool.tile([128, 2, L], BF, name="st", tag="st")
            for s in range(4):
                col0 = s * SUB
                nc.scalar.copy(st[:, :, col0:col0 + SUB],
                               dbuf[:, :, 1 + col0:1 + col0 + SUB])
                po = psum_o.tile([128, 2, SUB], F32, name="po", tag="po")
                for oh in range(2):
                    for k in range(2):
                        nc.tensor.matmul(po[:, oh, :],
                                         wo_sb[k][:, oh * 128:(oh + 1) * 128],
                                         st[:, k, col0:col0 + SUB],
                                         start=(k == 0), stop=(k == 1))
                osb = osb_pool.tile([128, 2, SUB], F32, name="osb", tag="osb")
                nc.scalar.copy(osb, po)
                skip = 1 if (c == 0 and s == 0) else 0
                dst = out[:, obase + col0 + skip:obase + col0 + SUB]
                nc.sync.dma_start(
                    out=dst.rearrange("(two p) n -> p two n", p=128),
                    in_=osb[:, :, skip:])

        # ---- software-pipelined emission: phase1(c+2) goes between the
        # tree chain of chunk c and the Wo phase of chunk c, so PE/ACT work
        # for c+2 isn't head-blocked behind Wo(c)'s cast dependency ----
        emit_phase1(0)
        emit_phase1(1)
        for c in range(NCHUNK):
            *_, dbuf_c = emit_tree_and_out(c)
            if c + 2 < NCHUNK:
                emit_phase1(c + 2)
            emit_out(c, dbuf_c)
            last_dbuf = dbuf_c

        # ---- final position T: out[p] = out[p-1]*a + b ----
        xl = singles.tile([128, 2, 1], BF, name="xl")
        nc.sync.dma_start(out=xl,
                          in_=xt[:, T - 1:T].rearrange("(k p) n -> p k n", p=128))
        al = singles.tile([128, 2, 1], F32, name="al")
        bl = singles.tile([128, 2, 1], F32, name="bl")
        for h in range(2):
            yzl = psum_y.tile([128, SUB], F32, name="yzl", tag=f"y{h}")[:, 0:1]
            for k in range(2):
                nc.tensor.matmul(yzl, wz_sb[k][:, h * 128:(h + 1) * 128],
                                 xl[:, k, :], start=(k == 0), stop=(k == 1))
            zl = singles.tile([128, 1], F32, name=f"zl{h}")
            nc.scalar.activation(zl, yzl, ActFn.Sigmoid,
                                 bias=bzp_sb[h][:, 0:1], scale=1.0)
            nc.scalar.activation(al[:, h, :], yzl, ActFn.Sigmoid,
                                 bias=bzn_sb[h][:, 0:1], scale=-1.0)
            yhl = psum_y.tile([128, SUB], F32, name="yhl", tag=f"y{h}")[:, 0:1]
            for k in range(2):
                nc.tensor.matmul(yhl, wh_sb[k][:, h * 128:(h + 1) * 128],
                                 xl[:, k, :], start=(k == 0), stop=(k == 1))
            nc.vector.scalar_tensor_tensor(bl[:, h, :], yhl, bh_sb[h][:, 0:1],
                                           zl, op0=AluOp.add, op1=AluOp.mult)
        dl = singles.tile([128, 2, 1], F32, name="dl")
        sl = singles.tile([128, 2, 1], BF, name="sl")
        nc.vector.tensor_tensor(dl, last_dbuf[:, :, L:L + 1], al, op=AluOp.mult)
        nc.vector.tensor_tensor(dl, dl, bl, op=AluOp.add)
        nc.scalar.copy(sl, dl)
        pol = psum_o.tile([128, 2, SUB], F32, name="pol", tag="po")[:, :, 0:1]
        for oh in range(2):
            for k in range(2):
                nc.tensor.matmul(pol[:, oh, :],
                                 wo_sb[k][:, oh * 128:(oh + 1) * 128],
                                 sl[:, k, :], start=(k == 0), stop=(k == 1))
        osl = singles.tile([128, 2, 1], F32, name="osl")
        nc.scalar.copy(osl, pol)
        nc.sync.dma_start(
            out=out[:, T - 1:T].rearrange("(two p) n -> p two n", p=128),
            in_=osl)

    nc.compile()
    return nc


_NC_CACHE = {}


def _get_nc():
    if "nc" not in _NC_CACHE:
        _NC_CACHE["nc"] = build_nc()
    return _NC_CACHE["nc"]


def _prepare_in_maps(xs, Wz, bz, Wh, bh, Wo, bo):
    xs = np.asarray(xs, np.float32)
    Wz = np.asarray(Wz, np.float32)
    bz = np.asarray(bz, np.float32)
    Wh = np.asarray(Wh, np.float32)
    bh = np.asarray(bh, np.float32)
    Wo = np.asarray(Wo, np.float32)

    wzt = np.ascontiguousarray(Wz.T).astype(BF16)
    wht = np.ascontiguousarray(Wh.T).astype(BF16)
    wot = np.ascontiguousarray(Wo.T).astype(BF16)
    bzp = np.ascontiguousarray(bz.reshape(H, 1))
    bzn = np.ascontiguousarray((-bz).reshape(H, 1))
    bhb = np.ascontiguousarray(bh.reshape(H, 1))

    in_maps = []
    for i in range(B):
        xti = np.ascontiguousarray(xs[i].T).astype(BF16)
        in_maps.append({
            "xt": xti, "wzt": wzt, "wht": wht, "wot": wot,
            "bzp": bzp, "bzn": bzn, "bhb": bhb,
        })
    return in_maps


def _assemble(res, bo):
    bo = np.asarray(bo, np.float32)
    return np.stack([np.asarray(res.results[i]["out"], np.float32).T + bo
                     for i in range(B)], axis=0)


def run_traced(xs, Wz, bz, Wh, bh, Wo, bo, trace=True):
    in_maps = _prepare_in_maps(xs, Wz, bz, Wh, bh, Wo, bo)
    res = run_bass_kernel_spmd(_get_nc(), in_maps, core_ids=list(range(B)),
                               trace=trace)
    return _assemble(res, bo), res


def kernel(xs, Wz, bz, Wh, bh, Wo, bo):
    in_maps = _prepare_in_maps(xs, Wz, bz, Wh, bh, Wo, bo)
    res = run_bass_kernel_spmd(_get_nc(), in_maps, core_ids=list(range(B)))
    return _assemble(res, bo)
